# revision 7
# baseline (speedup 1.0000x reference)
"""Trainium2 Bass kernel for bipartite cross-batch attention.

Reference computation (per full inputs):
  q  = LN(qx; gq,bq) @ Wq.T            -> [Bq, H, hd]
  k  = LN(kx; gk,bk) @ Wk.T            -> [Bk, Nk, H, hd]
  a  = softmax(q.k * hd^-0.5, axis=Nk) -> [Bq, Bk, H, Nk]
  w  = a.sum(H)                        -> [Bq, Bk, Nk]
  out= einsum('knc,qkn->qkc', kx, w)   -> [Bq, Bk, C]

Bq=128, Bk=128, Nk=256, C=1024, H=16, hd=64.

Distribution: shard Bk across the 8 cores (16 k-batches each). The softmax
axis is Nk, so every (q, k-batch) slab is fully core-local -- no collectives.
This splits the dominant K-projection (69 of 86 GFLOP) 8 ways, unlike the
Bq-sharding hint, which would replicate it on every core.

Host-side algebraic prep (exact reparameterizations, dtype aside):
  - gq/gk fold into the projection weights: (LN*g) @ W.T == LN @ (W*g).T.
  - bk drops: it shifts scores uniformly over Nk -> softmax-invariant.
  - bq folds into a per-output-channel bias added after the q projection.
  - hd^-0.5 folds into Wq.
  - LN's rstd (per key row) commutes past the k projection; it is applied to
    kxT columns before the matmul. The mean subtraction becomes a rank-1
    accumulating matmul with colsum(Wk') and (mean*rstd) rows.
Matmuls run in bf16 with f32 PSUM accumulation; softmax in f32.
"""

import numpy as np
import ml_dtypes

BF16 = ml_dtypes.bfloat16
H, C, HD = 16, 1024, 64
BQ, BK, NK = 128, 128, 256
NCORES = 8
BKL = BK // NCORES  # k-batches per core
EPS = 1e-5

_CACHE: dict = {}


def _build():
    from contextlib import ExitStack
    from concourse import bacc, tile, mybir

    f32 = mybir.dt.float32
    bf16 = mybir.dt.bfloat16
    Alu = mybir.AluOpType
    Act = mybir.ActivationFunctionType

    nc = bacc.Bacc("TRN2", target_bir_lowering=False, debug=False)

    # [b, p, i, n] = kx[b, n, i*128+p]  (c-major transposed layout)
    kxt_d = nc.dram_tensor("kxt", [BKL, 128, 8, NK], bf16, kind="ExternalInput").ap()
    # [b, p, j, c] = kx[b, j*128+p, c] (natural layout)
    kxn_d = nc.dram_tensor("kxn", [BKL, 128, 2, C], bf16, kind="ExternalInput").ap()
    qx_d = nc.dram_tensor("qx", [BQ, C], f32, kind="ExternalInput").ap()
    # [p, i, o] = Wq'[i*128+p, o]  with Wq'[c,o] = Wq[o,c]*gq[c]*hd^-0.5
    wq_d = nc.dram_tensor("wq", [128, 8, C], bf16, kind="ExternalInput").ap()
    wk_d = nc.dram_tensor("wk", [128, 8, C], bf16, kind="ExternalInput").ap()
    cneg_d = nc.dram_tensor("cneg", [1, C], bf16, kind="ExternalInput").ap()
    bqt_d = nc.dram_tensor("bqt", [128, 8], f32, kind="ExternalInput").ap()
    ones_d = nc.dram_tensor("ones", [1, 128], bf16, kind="ExternalInput").ap()
    id_d = nc.dram_tensor("ident", [128, 128], bf16, kind="ExternalInput").ap()
    out_d = nc.dram_tensor("out", [BKL, BQ, C], f32, kind="ExternalOutput").ap()

    with tile.TileContext(nc) as tc, ExitStack() as ctx:
        const = ctx.enter_context(tc.tile_pool(name="const", bufs=1))
        qpool = ctx.enter_context(tc.tile_pool(name="qpool", bufs=1))
        kt_p = ctx.enter_context(tc.tile_pool(name="kt", bufs=3))
        kn_p = ctx.enter_context(tc.tile_pool(name="kn", bufs=3))
        kts_p = ctx.enter_context(tc.tile_pool(name="kts", bufs=2))
        kj_p = ctx.enter_context(tc.tile_pool(name="kj", bufs=2))
        st_p = ctx.enter_context(tc.tile_pool(name="st", bufs=4))
        row_p = ctx.enter_context(tc.tile_pool(name="rows", bufs=2))
        ex_p = ctx.enter_context(tc.tile_pool(name="ex", bufs=4))
        den_p = ctx.enter_context(tc.tile_pool(name="den", bufs=6))
        w_p = ctx.enter_context(tc.tile_pool(name="w", bufs=2))
        out_p = ctx.enter_context(tc.tile_pool(name="outp", bufs=2))
        # PSUM: 8 banks total; each buf pads to one bank.
        pp_tp = ctx.enter_context(tc.tile_pool(name="pp_tp", bufs=2, space="PSUM"))
        pp_kp = ctx.enter_context(tc.tile_pool(name="pp_kp", bufs=2, space="PSUM"))
        pp_sc = ctx.enter_context(tc.tile_pool(name="pp_sc", bufs=2, space="PSUM"))
        pp_av = ctx.enter_context(tc.tile_pool(name="pp_av", bufs=2, space="PSUM"))

        # ---- constants ----
        wq_t = const.tile([128, 8, C], bf16)
        nc.sync.dma_start(wq_t[:], wq_d[:])
        wk_t = const.tile([128, 8, C], bf16)
        nc.sync.dma_start(wk_t[:], wk_d[:])
        cneg_t = const.tile([1, C], bf16)
        nc.sync.dma_start(cneg_t[:], cneg_d[:])
        bqt_t = const.tile([128, 8], f32)
        nc.sync.dma_start(bqt_t[:], bqt_d[:])
        ones_t = const.tile([1, 128], bf16)
        nc.sync.dma_start(ones_t[:], ones_d[:])
        id_t = const.tile([128, 128], bf16)
        nc.sync.dma_start(id_t[:], id_d[:])
        eps_t = const.tile([128, 1], f32)
        nc.vector.memset(eps_t[:], EPS)

        # ---- Q path (once) ----
        qx_t = qpool.tile([BQ, C], f32)
        nc.sync.dma_start(qx_t[:], qx_d[:])
        qst = qpool.tile([128, 2, 6], f32)
        nc.vector.bn_stats(qst[:, 0, :], qx_t[:, 0:512])
        nc.vector.bn_stats(qst[:, 1, :], qx_t[:, 512:1024])
        qmv = qpool.tile([128, 2], f32)
        nc.vector.bn_aggr(qmv[:], qst[:])
        qsd = qpool.tile([128, 1], f32)
        nc.scalar.activation(qsd[:], qmv[:, 1:2], Act.Sqrt, bias=eps_t[:])
        qrs = qpool.tile([128, 1], f32)
        nc.vector.reciprocal(qrs[:], qsd[:])
        lnq = qpool.tile([BQ, C], bf16)
        nc.vector.tensor_scalar(
            lnq[:], qx_t[:], qmv[:, 0:1], qrs[:], op0=Alu.subtract, op1=Alu.mult
        )
        lnqT = qpool.tile([128, 8, 128], bf16)
        for j in range(8):
            tp = pp_tp.tile([128, NK], bf16, tag="tp")
            nc.tensor.transpose(tp[:, 0:128], lnq[:, j * 128 : (j + 1) * 128], id_t[:])
            nc.any.tensor_copy(lnqT[:, j, :], tp[:, 0:128])
        qT = qpool.tile([128, 8, 128], bf16)  # [o%128, o//128, q]
        for j in range(8):
            qp = pp_kp.tile([128, NK], f32, tag="kp")
            for i in range(8):
                nc.tensor.matmul(
                    qp[:, 0:128],
                    wq_t[:, i, j * 128 : (j + 1) * 128],
                    lnqT[:, i, :],
                    start=(i == 0),
                    stop=(i == 7),
                )
            nc.vector.tensor_scalar(
                qT[:, j, :], qp[:, 0:128], bqt_t[:, j : j + 1], None, op0=Alu.add
            )

        # ---- K loop ----
        for b in range(BKL):
            kT_t = kt_p.tile([128, 8, NK], bf16, tag="kt")
            nc.sync.dma_start(kT_t[:], kxt_d[b])
            kn_t = kn_p.tile([128, 2, C], bf16, tag="kn")
            nc.sync.dma_start(kn_t[:], kxn_d[b])

            # per-row LN stats (rows n on partitions), then rows [1, 256]
            ms_row = row_p.tile([1, NK], bf16, tag="msr")
            s_row = row_p.tile([1, NK], bf16, tag="sr")
            for j in range(2):
                st6 = st_p.tile([128, 2, 6], f32, tag="st6")
                nc.vector.bn_stats(st6[:, 0, :], kn_t[:, j, 0:512])
                nc.vector.bn_stats(st6[:, 1, :], kn_t[:, j, 512:1024])
                mv = st_p.tile([128, 2], f32, tag="mv")
                nc.vector.bn_aggr(mv[:], st6[:])
                sd = st_p.tile([128, 1], f32, tag="sd")
                nc.scalar.activation(sd[:], mv[:, 1:2], Act.Sqrt, bias=eps_t[:])
                rs = st_p.tile([128, 1], f32, tag="rs")
                nc.vector.reciprocal(rs[:], sd[:])
                ms = st_p.tile([128, 1], f32, tag="ms")
                nc.vector.tensor_scalar(ms[:], mv[:, 0:1], rs[:], None, op0=Alu.mult)
                msb = st_p.tile([128, 1], bf16, tag="msb")
                nc.any.tensor_copy(msb[:], ms[:])
                rsb = st_p.tile([128, 1], bf16, tag="rsb")
                nc.any.tensor_copy(rsb[:], rs[:])
                tp1 = pp_tp.tile([128, NK], bf16, tag="tp")
                nc.tensor.transpose(tp1[0:1, 0:128], msb[:], id_t[:])
                nc.any.tensor_copy(ms_row[0:1, j * 128 : (j + 1) * 128], tp1[0:1, 0:128])
                tp2 = pp_tp.tile([128, NK], bf16, tag="tp")
                nc.tensor.transpose(tp2[0:1, 0:128], rsb[:], id_t[:])
                nc.any.tensor_copy(s_row[0:1, j * 128 : (j + 1) * 128], tp2[0:1, 0:128])

            # broadcast rstd down partitions: s_bcast[c, n] = rstd[n]
            sbp = pp_tp.tile([128, NK], f32, tag="tp")
            nc.tensor.matmul(sbp[:], ones_t[:], s_row[0:1, :], start=True, stop=True)

            # scale kxT columns by rstd (rest of LN folds into the matmul)
            kTs = kts_p.tile([128, 8, NK], bf16, tag="kts")
            for i in range(8):
                nc.vector.scalar_tensor_tensor(
                    kTs[:, i, :], kT_t[:, i, :], 1.0, sbp[:],
                    op0=Alu.mult, op1=Alu.mult,
                )

            # K projection -> kjp[o%128, o//128, n]  (transposed, per-head slices)
            kjp = kj_p.tile([128, 8, NK], bf16, tag="kj")
            for j in range(8):
                kpp = pp_kp.tile([128, NK], f32, tag="kp")
                for i in range(8):
                    nc.tensor.matmul(
                        kpp[:], wk_t[:, i, j * 128 : (j + 1) * 128], kTs[:, i, :],
                        start=(i == 0), stop=False,
                    )
                nc.tensor.matmul(
                    kpp[:], cneg_t[0:1, j * 128 : (j + 1) * 128], ms_row[0:1, :],
                    start=False, stop=True,
                )
                nc.any.tensor_copy(kjp[:, j, :], kpp[:])

            # scores + softmax + head-sum
            w_acc = w_p.tile([BQ, NK], f32, tag="wacc")
            for h in range(H):
                j, off = h // 2, (h % 2) * 64
                scp = pp_sc.tile([BQ, NK], f32, tag="sc")
                nc.tensor.matmul(
                    scp[:], qT[off : off + 64, j, :], kjp[off : off + 64, j, :],
                    start=True, stop=True,
                )
                ex = ex_p.tile([BQ, NK], f32, tag="ex")
                den = den_p.tile([BQ, 1], f32, tag="den")
                nc.scalar.activation(ex[:], scp[:], Act.Exp, accum_out=den[:])
                idn = den_p.tile([BQ, 1], f32, tag="idn")
                nc.vector.reciprocal(idn[:], den[:])
                if h == 0:
                    nc.vector.tensor_scalar(
                        w_acc[:], ex[:], idn[:], None, op0=Alu.mult
                    )
                else:
                    nc.vector.scalar_tensor_tensor(
                        w_acc[:], ex[:], idn[:], w_acc[:],
                        op0=Alu.mult, op1=Alu.add,
                    )

            w_bf = w_p.tile([BQ, NK], bf16, tag="wbf")
            nc.any.tensor_copy(w_bf[:], w_acc[:])
            wT = w_p.tile([128, 2, 128], bf16, tag="wT")
            for t in range(2):
                wtp = pp_tp.tile([128, NK], bf16, tag="tp")
                nc.tensor.transpose(
                    wtp[:, 0:128], w_bf[:, t * 128 : (t + 1) * 128], id_t[:]
                )
                nc.any.tensor_copy(wT[:, t, :], wtp[:, 0:128])

            # AV: out[q, c] = sum_n w[q, n] kx[n, c]
            out_sb = out_p.tile([BQ, C], f32, tag="osb")
            for m in range(2):
                avp = pp_av.tile([BQ, 512], f32, tag="av")
                for t in range(2):
                    nc.tensor.matmul(
                        avp[:], wT[:, t, :], kn_t[:, t, m * 512 : (m + 1) * 512],
                        start=(t == 0), stop=(t == 1),
                    )
                nc.any.tensor_copy(out_sb[:, m * 512 : (m + 1) * 512], avp[:])
            nc.sync.dma_start(out_d[b], out_sb[:])

    nc.compile()
    return nc


def _prep(qx, kx, gq, bq, gk, bk, Wq, Wk):
    scale = HD ** -0.5
    qx_h = np.ascontiguousarray(qx[:, 0, :], dtype=np.float32)
    Wqp = (Wq * gq[None, :]).T.astype(np.float32) * scale  # [c, o]
    Wkp = (Wk * gk[None, :]).T.astype(np.float32)  # [c, o]
    wq_h = np.ascontiguousarray(
        Wqp.reshape(8, 128, C).transpose(1, 0, 2)).astype(BF16)
    wk_h = np.ascontiguousarray(
        Wkp.reshape(8, 128, C).transpose(1, 0, 2)).astype(BF16)
    cneg_h = (-Wkp.sum(axis=0)).reshape(1, C).astype(BF16)
    bq_h = (scale * (bq[None, :] @ Wq.T)).reshape(8, 128).T.astype(np.float32)
    bq_h = np.ascontiguousarray(bq_h)
    ones_h = np.ones((1, 128), dtype=BF16)
    id_h = np.eye(128, dtype=np.float32).astype(BF16)

    shared = dict(qx=qx_h, wq=wq_h, wk=wk_h, cneg=cneg_h, bqt=bq_h,
                  ones=ones_h, ident=id_h)
    in_maps = []
    for i in range(NCORES):
        kxl = np.asarray(kx[i * BKL : (i + 1) * BKL], dtype=np.float32)
        kxt_h = np.ascontiguousarray(
            kxl.transpose(0, 2, 1).reshape(BKL, 8, 128, NK).transpose(0, 2, 1, 3)
        ).astype(BF16)
        kxn_h = np.ascontiguousarray(
            kxl.reshape(BKL, 2, 128, C).transpose(0, 2, 1, 3)
        ).astype(BF16)
        in_maps.append(dict(kxt=kxt_h, kxn=kxn_h, **shared))
    return in_maps


def kernel(qx, kx, gq, bq, gk, bk, Wq, Wk):
    from concourse.bass_utils import run_bass_kernel_spmd

    if "nc" not in _CACHE:
        _CACHE["nc"] = _build()
    nc = _CACHE["nc"]
    in_maps = _prep(qx, kx, gq, bq, gk, bk, Wq, Wk)
    res = run_bass_kernel_spmd(nc, in_maps, core_ids=list(range(NCORES)))
    full = np.concatenate([r["out"] for r in res.results], axis=0)  # [Bk, Bq, C]
    return np.ascontiguousarray(full.transpose(1, 0, 2))  # [Bq, Bk, C]
